# revision 1
# baseline (speedup 1.0000x reference)
"""Trainium2 Bass kernel for EntropyModelSoS quantize+dequantize.

reference semantics (all f32):
    lv  = sort(levels)                       # [64]
    mid = (lv[:-1] + lv[1:]) * 0.5           # [63] decision boundaries
    sym = searchsorted(mid, x)               # = sum_j [x > mid_j], int
    deq = lv[sym] + means                    # gather + per-channel mean

Device algorithm (per core, exact):
    sym  = sum_j (x > mid_j)                   63 fused compare-accumulate
    deqv = sum_j (x > mid_j) * (lv[j+1]-lv[j]) 63 weighted compares + adds
    deq  = deqv + (lv[0] + mean_c)             per-partition scalar add

Sharding: data-parallel along batch (16 batches -> 2 per core); the level
table is tiny and baked into the program as immediates.
"""

import sys

sys.path.insert(0, "/opt/trn_rl_repo")

import numpy as np

B, C, H, W = 16, 192, 64, 64
L = 64
N_CORES = 8
B_PER_CORE = B // N_CORES          # 2
ROWS = B_PER_CORE * C              # 384 partition rows per core
COLS = H * W                       # 4096
PB = 128                           # partition block
FB = 2048                          # free-dim block

_CACHE = {}


def _build_program(mid, deltas):
    """Build + compile the SPMD Bass program. mid/deltas: np.float32[63]."""
    import concourse.tile as tile
    from concourse import bacc, mybir

    nc = bacc.Bacc(
        "TRN2",
        target_bir_lowering=False,
        debug=False,
        enable_asserts=False,
        num_devices=N_CORES,
    )
    f32 = mybir.dt.float32
    i32 = mybir.dt.int32
    gt = mybir.AluOpType.is_gt
    add = mybir.AluOpType.add
    mult = mybir.AluOpType.mult

    x_ap = nc.dram_tensor("x", (ROWS, COLS), f32, kind="ExternalInput").ap()
    mlv_ap = nc.dram_tensor("meanlv", (ROWS, 1), f32, kind="ExternalInput").ap()
    sym_ap = nc.dram_tensor("sym", (ROWS, COLS), i32, kind="ExternalOutput").ap()
    deq_ap = nc.dram_tensor("deq", (ROWS, COLS), f32, kind="ExternalOutput").ap()

    m = [float(v) for v in mid]
    d = [float(v) for v in deltas]

    with tile.TileContext(nc) as tc:
        with tc.tile_pool(name="mlv", bufs=3) as mlvpool, \
             tc.tile_pool(name="xp", bufs=3) as xpool, \
             tc.tile_pool(name="accS", bufs=2) as spool, \
             tc.tile_pool(name="accD", bufs=2) as dpool, \
             tc.tile_pool(name="wp", bufs=2) as wpool, \
             tc.tile_pool(name="osym", bufs=2) as sympool, \
             tc.tile_pool(name="odeq", bufs=2) as deqpool:
            for pb in range(ROWS // PB):
                mlv_t = mlvpool.tile([PB, 1], f32)
                nc.sync.dma_start(mlv_t[:], mlv_ap[pb * PB:(pb + 1) * PB, :])
                for fb in range(COLS // FB):
                    rs = slice(pb * PB, (pb + 1) * PB)
                    cs = slice(fb * FB, (fb + 1) * FB)
                    xt = xpool.tile([PB, FB], f32)
                    nc.sync.dma_start(xt[:], x_ap[rs, cs])

                    accS = spool.tile([PB, FB], f32)
                    accD = dpool.tile([PB, FB], f32)
                    w = wpool.tile([PB, FB], f32)

                    # j = 0 initializes both accumulators
                    nc.vector.tensor_scalar(accS[:], xt[:], m[0], None, gt)
                    nc.vector.tensor_scalar(accD[:], xt[:], m[0], d[0], gt, mult)
                    for j in range(1, 63):
                        nc.vector.scalar_tensor_tensor(
                            accS[:], xt[:], m[j], accS[:], gt, add
                        )
                        nc.vector.tensor_scalar(w[:], xt[:], m[j], d[j], gt, mult)
                        nc.vector.tensor_tensor(accD[:], accD[:], w[:], add)

                    symi = sympool.tile([PB, FB], i32)
                    nc.vector.tensor_copy(symi[:], accS[:])
                    deqt = deqpool.tile([PB, FB], f32)
                    nc.vector.tensor_scalar(deqt[:], accD[:], mlv_t[:], None, add)

                    nc.sync.dma_start(sym_ap[rs, cs], symi[:])
                    nc.sync.dma_start(deq_ap[rs, cs], deqt[:])

    nc.compile()
    return nc


def _prep(levels, means):
    lv = np.sort(np.asarray(levels, dtype=np.float32))
    mid = ((lv[:-1] + lv[1:]) * np.float32(0.5)).astype(np.float32)
    deltas = (lv[1:] - lv[:-1]).astype(np.float32)
    mean_c = np.asarray(means, dtype=np.float32).reshape(C)
    # per-partition row value: lv[0] + mean[channel]; rows are (b, c) pairs
    mlv = (lv[0] + mean_c).astype(np.float32)
    mlv = np.tile(mlv, B_PER_CORE).reshape(ROWS, 1)
    return lv, mid, deltas, mlv


def get_program(levels, means):
    lv, mid, deltas, mlv = _prep(levels, means)
    key = (mid.tobytes(), deltas.tobytes())
    if key not in _CACHE:
        _CACHE[key] = _build_program(mid, deltas)
    return _CACHE[key], mlv


def make_in_maps(x, mlv):
    x = np.asarray(x, dtype=np.float32)
    return [
        {
            "x": np.ascontiguousarray(
                x[k * B_PER_CORE:(k + 1) * B_PER_CORE].reshape(ROWS, COLS)
            ),
            "meanlv": mlv,
        }
        for k in range(N_CORES)
    ]


def assemble(results):
    sym = np.concatenate(
        [results[k]["sym"].reshape(B_PER_CORE, C, H, W) for k in range(N_CORES)],
        axis=0,
    ).astype(np.int32)
    deq = np.concatenate(
        [results[k]["deq"].reshape(B_PER_CORE, C, H, W) for k in range(N_CORES)],
        axis=0,
    ).astype(np.float32)
    return sym, deq


def kernel(x, levels, means):
    from concourse.bass_utils import run_bass_kernel_spmd

    nc, mlv = get_program(levels, means)
    in_maps = make_in_maps(x, mlv)
    res = run_bass_kernel_spmd(nc, in_maps, list(range(N_CORES)))
    return assemble(res.results)


# revision 7
# speedup vs baseline: 23.4747x; 23.4747x over previous
"""Trainium2 Bass kernel for EntropyModelSoS quantize+dequantize.

reference semantics (all f32):
    lv  = sort(levels)                       # [64]
    mid = (lv[:-1] + lv[1:]) * 0.5           # [63] decision boundaries
    sym = searchsorted(mid, x)               # = sum_j [x > mid_j], int
    deq = lv[sym] + means                    # gather + per-channel mean

Device algorithm (per core): a single packed thermometer chain using a
custom fused DVE op  acc' = (x > m_j)*w_j + acc  with
    w_j = 32 + (T[j+1] - T[j]),   T[s] = round((lv[s]-lv[0])*2^13)/2^13
After 63 steps  acc = 32*sym + T[sym]  exactly (every partial sum is a
multiple of 2^-13 below 2^11, hence exact in f32; T[sym] < 16 < 32/2 so
the fields separate).  Extraction:
    sym  = round(acc/32)            (frac < 0.5)
    deq  = (acc - 32*sym) + (lv[0] + mean_c)   -> error <= 2^-14 + 1 ulp

sym is bit-exact vs the reference (including ties: is_gt matches
searchsorted side='left').  Sharding: data-parallel along batch
(16 batches -> 2 per core); level constants are baked as immediates.
"""

import sys

sys.path.insert(0, "/opt/trn_rl_repo")

import numpy as np

B, C, H, W = 16, 192, 64, 64
L = 64
N_CORES = 8
B_PER_CORE = B // N_CORES          # 2
ROWS = B_PER_CORE * C              # 384 partition rows per core
COLS = H * W                       # 4096
PB = 128                           # partition block
FB = 2048                          # free-dim block

SEP = 32.0                         # sym field separation (power of two)
GRID = 2.0 ** 13                   # dequant quantization grid (2^-13)

_CACHE = {}
_OP = None


def _get_dve_op():
    """Register (once) the fused op: out = (in0 > s0)*s1 + in1."""
    global _OP
    if _OP is not None:
        return _OP
    import concourse.dve_ops as dve_ops
    from concourse.dve_spec import C0, C1, Spec, Src0, Src1, _has_src1, lower
    from concourse.dve_uop import DveOpSpec

    name = "STEP_W_ACC_ANT"
    for op in dve_ops.OPS:
        if op.name == name:
            _OP = op
            return op

    def _ref(in0, in1, s0, s1, imm2):
        return (
            (in0.astype(np.float32) > np.float32(s0)).astype(np.float32)
            * np.float32(s1)
            + in1
        ).astype(np.float32)

    spec = Spec(body=(Src0 > C0) * C1 + Src1, reference=_ref)
    opcode = dve_ops._CUSTOM_DVE_ROW_BASE + len(dve_ops.OPS)
    assert opcode < 0x20
    dve_ops._SUB_OPCODE_FOR_NAME[name] = opcode
    shas = {}
    for ver in ("v3", "v4"):
        s = DveOpSpec(
            name=name, opcode=opcode, uops=lower(spec, ver=ver),
            rd1_en=_has_src1(spec),
        )
        shas[ver] = s.sha(ver)
    op = dve_ops.DveOp(name, spec, subdim=False, uops_sha=shas)
    dve_ops.OPS.append(op)
    dve_ops.CUSTOM_DVE_SPECS[name] = spec
    _OP = op
    return op


def _build_program(mid, weights, repeat=1):
    """Build + compile the SPMD Bass program.

    mid: np.float32[63] boundaries; weights: np.float32[63] packed
    increments (SEP + T-deltas). repeat>1 re-runs the computation for
    differential timing."""
    import concourse.tile as tile
    from concourse import bacc, mybir

    dve_op = _get_dve_op()

    nc = bacc.Bacc(
        "TRN2",
        target_bir_lowering=False,
        debug=False,
        enable_asserts=False,
        num_devices=N_CORES,
    )
    f32 = mybir.dt.float32
    i32 = mybir.dt.int32
    gt = mybir.AluOpType.is_gt
    add = mybir.AluOpType.add
    mult = mybir.AluOpType.mult

    x_ap = nc.dram_tensor("x", (ROWS, COLS), f32, kind="ExternalInput").ap()
    mlv_ap = nc.dram_tensor("meanlv", (ROWS, 1), f32, kind="ExternalInput").ap()
    sym_ap = nc.dram_tensor("sym", (ROWS, COLS), i32, kind="ExternalOutput").ap()
    deq_ap = nc.dram_tensor("deq", (ROWS, COLS), f32, kind="ExternalOutput").ap()

    m = [float(v) for v in mid]
    wgt = [float(v) for v in weights]

    with tile.TileContext(nc) as tc:
        with tc.tile_pool(name="mlv", bufs=3) as mlvpool, \
             tc.tile_pool(name="xp", bufs=3) as xpool, \
             tc.tile_pool(name="acc", bufs=2) as accpool, \
             tc.tile_pool(name="symf", bufs=2) as symfpool, \
             tc.tile_pool(name="osym", bufs=2) as sympool, \
             tc.tile_pool(name="odeq", bufs=2) as deqpool:
            for rep in range(repeat):
              for pb in range(ROWS // PB):
                mlv_t = mlvpool.tile([PB, 1], f32)
                nc.sync.dma_start(mlv_t[:], mlv_ap[pb * PB:(pb + 1) * PB, :])
                for fb in range(COLS // FB):
                    rs = slice(pb * PB, (pb + 1) * PB)
                    cs = slice(fb * FB, (fb + 1) * FB)
                    xt = xpool.tile([PB, FB], f32)
                    nc.sync.dma_start(xt[:], x_ap[rs, cs])

                    acc = accpool.tile([PB, FB], f32)
                    # j=0 initializes: acc = (x > m0)*w0
                    nc.vector.tensor_scalar(acc[:], xt[:], m[0], wgt[0], gt, mult)
                    for j in range(1, 63):
                        nc.vector._custom_dve(
                            dve_op, out=acc[:], in0=xt[:], in1=acc[:],
                            s0=m[j], s1=wgt[j],
                        )

                    symi = sympool.tile([PB, FB], i32)
                    nc.vector.tensor_scalar(symi[:], acc[:], 1.0 / SEP, None, mult)
                    symf = symfpool.tile([PB, FB], f32)
                    nc.vector.tensor_copy(symf[:], symi[:])
                    deqt = deqpool.tile([PB, FB], f32)
                    # (symf*-SEP + acc) = T[sym]; then + (lv0+mean) per row
                    nc.vector.scalar_tensor_tensor(
                        deqt[:], symf[:], -SEP, acc[:], mult, add
                    )
                    nc.vector.tensor_scalar(deqt[:], deqt[:], mlv_t[:], None, add)

                    nc.sync.dma_start(sym_ap[rs, cs], symi[:])
                    nc.sync.dma_start(deq_ap[rs, cs], deqt[:])

    nc.compile()
    return nc


def _prep(levels, means):
    lv = np.sort(np.asarray(levels, dtype=np.float32))
    mid = ((lv[:-1] + lv[1:]) * np.float32(0.5)).astype(np.float32)
    # T[s] = (lv[s]-lv[0]) rounded to the 2^-13 grid (exact in f32)
    t = np.round((lv.astype(np.float64) - np.float64(lv[0])) * GRID) / GRID
    assert t[-1] < SEP / 2 - 0.05, f"level span {t[-1]} too large for SEP={SEP}"
    assert (63 * SEP + t[-1]) * GRID < 2 ** 24, "packed sums exceed f32 exactness"
    weights = (np.float64(SEP) + np.diff(t)).astype(np.float32)
    # exactness of each weight: SEP + delta is a multiple of 2^-13 < 2^6
    mean_c = np.asarray(means, dtype=np.float32).reshape(C)
    mlv = (lv[0] + mean_c).astype(np.float32)
    mlv = np.tile(mlv, B_PER_CORE).reshape(ROWS, 1)
    return lv, mid, weights, mlv


def get_program(levels, means):
    lv, mid, weights, mlv = _prep(levels, means)
    key = (mid.tobytes(), weights.tobytes())
    if key not in _CACHE:
        _CACHE[key] = _build_program(mid, weights)
    return _CACHE[key], mlv


def make_in_maps(x, mlv):
    x = np.asarray(x, dtype=np.float32)
    return [
        {
            "x": np.ascontiguousarray(
                x[k * B_PER_CORE:(k + 1) * B_PER_CORE].reshape(ROWS, COLS)
            ),
            "meanlv": mlv,
        }
        for k in range(N_CORES)
    ]


def assemble(results):
    sym = np.concatenate(
        [results[k]["sym"].reshape(B_PER_CORE, C, H, W) for k in range(N_CORES)],
        axis=0,
    ).astype(np.int32)
    deq = np.concatenate(
        [results[k]["deq"].reshape(B_PER_CORE, C, H, W) for k in range(N_CORES)],
        axis=0,
    ).astype(np.float32)
    return sym, deq


def _run_once(x, levels, means):
    from concourse.bass_utils import run_bass_kernel_spmd

    nc, mlv = get_program(levels, means)
    in_maps = make_in_maps(x, mlv)
    res = run_bass_kernel_spmd(nc, in_maps, list(range(N_CORES)))
    return assemble(res.results)


def _run_subprocess(x, levels, means):
    """Fallback: run in a fresh process (a crashed axon/device session can
    poison the current process's PJRT client)."""
    import os
    import subprocess
    import tempfile

    kdir = os.path.dirname(os.path.abspath(__file__))
    with tempfile.TemporaryDirectory() as td:
        np.savez(os.path.join(td, "in.npz"), x=x, levels=levels, means=means)
        code = (
            "import sys, numpy as np; sys.path.insert(0, %r); "
            "import kernel as K; d = dict(np.load(%r)); "
            "s, q = K._run_once(d['x'], d['levels'], d['means']); "
            "np.savez(%r, sym=s, deq=q)"
            % (kdir, os.path.join(td, "in.npz"), os.path.join(td, "out.npz"))
        )
        subprocess.run([sys.executable, "-c", code], check=True, cwd=td)
        out = np.load(os.path.join(td, "out.npz"))
        return out["sym"].astype(np.int32), out["deq"].astype(np.float32)


def kernel(x, levels, means):
    import time as _time

    last_err = None
    for attempt in range(2):
        try:
            return _run_once(x, levels, means)
        except Exception as e:  # device/session crash: retry once in-process
            last_err = e
            _time.sleep(15)
    for attempt in range(3):
        try:
            return _run_subprocess(x, levels, means)
        except Exception as e:
            last_err = e
            _time.sleep(20)
    raise last_err
